# revision 1
# baseline (speedup 1.0000x reference)
"""KDE on a 20^3 grid, distributed across 8 TRN2 NeuronCores.

Math: kde[a] = sum_b K[a,b] * p[b], K[a,b] = coef * exp(-0.5 * d2[a,b]),
d2[a,b] = (x_a - x_b)^T A (x_a - x_b), then output = kde / sum(kde).
(coef cancels in the normalization, so it is never computed.)

Device algorithm (per core, rows sharded 8 ways -> 1000 rows/core):
  tile layout: partitions = b (kernel-source points, 63 chunks of 128 padded
  to 8064), free dim = i (this core's rows, 2 blocks of 500).
  d2[b,i] = q_b + q_i - 2*GA_i . x_b   with GA = x @ A, q = rowsum(x*GA)
  - one k=8 fp16 matmul per tile computes q_i - 2*GA_i.x_b
    (fp16 hi/lo split of -2*GA and q gives ~22-bit effective mantissa;
     the centered coords +-9.5, +-8.5, ... are exact in fp16)
  - ScalarE: E = exp(-0.5*in + bias_b), bias_b = -0.5*q_b  (per-partition)
  - accumulating matmul with p_b as stationary weights -> kde block [1, 500]
  - AllReduce the scalar normalizer, multiply by reciprocal, DMA out.
"""

import numpy as np

GRID = (20, 20, 20)
N = 8000
NCORES = 8
ROWS = N // NCORES          # 1000 rows per core
NCH = 63                    # b chunks of 128
NPAD = NCH * 128            # 8064
NBLK = 2
BLK = ROWS // NBLK          # 500

_PROGRAM = None


def _build_program(collective=True, repeat=1, num_devices=NCORES):
    from contextlib import ExitStack

    import concourse.bacc as bacc
    import concourse.mybir as mybir
    import concourse.tile as tile

    f32 = mybir.dt.float32
    f16 = mybir.dt.float16

    nc = bacc.Bacc(
        "TRN2",
        target_bir_lowering=False,
        debug=False,
        num_devices=num_devices,
    )

    lc_d = nc.dram_tensor("lc", [8, NPAD], f16, kind="ExternalInput").ap()
    csq_d = nc.dram_tensor("csq", [9, NPAD], f32, kind="ExternalInput").ap()
    cov9_d = nc.dram_tensor("cov9", [9, 1], f32, kind="ExternalInput").ap()
    cov3_d = nc.dram_tensor("cov3", [3, 3], f32, kind="ExternalInput").ap()
    ctl_d = nc.dram_tensor("ctl", [3, ROWS], f32, kind="ExternalInput").ap()
    csql_d = nc.dram_tensor("csql", [9, ROWS], f32, kind="ExternalInput").ap()
    pcol_d = nc.dram_tensor("pcol", [128, NCH], f32, kind="ExternalInput").ap()
    out_d = nc.dram_tensor("out", [1, ROWS], f32, kind="ExternalOutput").ap()

    with tile.TileContext(nc) as tc, ExitStack() as ctx:
        from contextlib import ExitStack as _ES

        const = ctx.enter_context(tc.tile_pool(name="const", bufs=1))
        work = ctx.enter_context(tc.tile_pool(name="work", bufs=4))
        pre_ctx = _ES()
        psum_pre = pre_ctx.enter_context(
            tc.tile_pool(name="psum_pre", bufs=1, space="PSUM")
        )
        dram = ctx.enter_context(tc.tile_pool(name="dram", bufs=1, space="DRAM"))

        # ---- input loads ----
        lc_sb = const.tile([8, NPAD], f16)
        nc.sync.dma_start(out=lc_sb[:], in_=lc_d[:])
        csq_sb = const.tile([9, NPAD], f32)
        nc.sync.dma_start(out=csq_sb[:], in_=csq_d[:])
        cov9 = const.tile([9, 1], f32)
        nc.sync.dma_start(out=cov9[:], in_=cov9_d[:])
        cov3 = const.tile([3, 3], f32)
        nc.sync.dma_start(out=cov3[:], in_=cov3_d[:])
        ctl_sb = const.tile([3, ROWS], f32)
        nc.sync.dma_start(out=ctl_sb[:], in_=ctl_d[:])
        csql_sb = const.tile([9, ROWS], f32)
        nc.sync.dma_start(out=csql_sb[:], in_=csql_d[:])
        pcol32 = const.tile([128, NCH], f32)
        nc.sync.dma_start(out=pcol32[:], in_=pcol_d[:])
        pcol16 = const.tile([128, NCH], f16)
        nc.vector.tensor_copy(pcol16[:], pcol32[:])

        # ---- local row factors: g2 = -2*GA_loc^T [3,ROWS], qloc [1,ROWS] ----
        g2 = const.tile([3, ROWS], f32)
        qloc = const.tile([1, ROWS], f32)
        for blk in range(NBLK):
            sl = slice(blk * BLK, (blk + 1) * BLK)
            gp = psum_pre.tile([3, BLK], f32)
            nc.tensor.matmul(
                gp[:], lhsT=cov3[:], rhs=ctl_sb[:, sl], start=True, stop=True
            )
            nc.vector.tensor_scalar_mul(g2[:, sl], gp[:], -2.0)
            qlp = psum_pre.tile([1, BLK], f32)
            nc.tensor.matmul(
                qlp[:], lhsT=cov9[:], rhs=csql_sb[:, sl], start=True, stop=True
            )
            nc.vector.tensor_copy(qloc[0:1, sl], qlp[:])

        # fp16 hi/lo split (compute engines need partition base 0; assemble
        # the 8-row factor tile with DMAs, which can write any partition)
        ghi = const.tile([3, ROWS], f16)
        nc.vector.tensor_copy(ghi[:], g2[:])
        ghi32 = const.tile([3, ROWS], f32)
        nc.vector.tensor_copy(ghi32[:], ghi[:])
        glo = const.tile([3, ROWS], f16)
        nc.vector.tensor_sub(glo[:], g2[:], ghi32[:])
        qhi = const.tile([1, ROWS], f16)
        nc.vector.tensor_copy(qhi[:], qloc[:])
        qhi32 = const.tile([1, ROWS], f32)
        nc.vector.tensor_copy(qhi32[:], qhi[:])
        qlo = const.tile([1, ROWS], f16)
        nc.vector.tensor_sub(qlo[:], qloc[:], qhi32[:])

        rfac = const.tile([8, ROWS], f16)
        nc.sync.dma_start(out=rfac[0:3, :], in_=ghi[:])
        nc.sync.dma_start(out=rfac[3:4, :], in_=qhi[:])
        nc.sync.dma_start(out=rfac[4:7, :], in_=glo[:])
        nc.sync.dma_start(out=rfac[7:8, :], in_=qlo[:])

        # ---- bias column: qb[lane, chunk] = q of source point b, times -0.5.
        # Converted in column groups so the first ACT only waits on group 0.
        qbias = const.tile([128, NCH], f32)
        QG = 16
        for g0 in range(0, NCH, QG):
            g1 = min(g0 + QG, NCH)
            qp = psum_pre.tile([128, QG], f32, tag="qp", bufs=2)
            for c in range(g0, g1):
                nc.tensor.matmul(
                    qp[:, c - g0 : c - g0 + 1],
                    lhsT=csq_sb[:, c * 128 : (c + 1) * 128],
                    rhs=cov9[:],
                    start=True,
                    stop=True,
                )
            nc.vector.tensor_scalar_mul(qbias[:, g0:g1], qp[:, : g1 - g0], -0.5)

        # free the precompute PSUM banks for deeper main-loop buffering
        pre_ctx.close()
        psum_dp = ctx.enter_context(tc.tile_pool(name="psum_dp", bufs=4, space="PSUM"))
        psum_kp = ctx.enter_context(tc.tile_pool(name="psum_kp", bufs=2, space="PSUM"))

        # ---- main loop ----
        kde_sb = const.tile([1, ROWS], f32)
        rep_ctx = tc.For_i(0, repeat, 1) if repeat > 1 else None
        if rep_ctx is not None:
            rep_ctx.__enter__()
        for blk in range(NBLK):
            sl = slice(blk * BLK, (blk + 1) * BLK)
            kp = psum_kp.tile([1, BLK], f32)
            for c in range(NCH):
                dp = psum_dp.tile([128, BLK], f32)
                nc.tensor.matmul(
                    dp[:],
                    lhsT=lc_sb[:, c * 128 : (c + 1) * 128],
                    rhs=rfac[:, sl],
                    start=True,
                    stop=True,
                )
                ek = work.tile([128, BLK], f16, tag="ek")
                nc.scalar.activation(
                    ek[:],
                    dp[:],
                    mybir.ActivationFunctionType.Exp,
                    bias=qbias[:, c : c + 1],
                    scale=-0.5,
                )
                nc.tensor.matmul(
                    kp[:],
                    lhsT=pcol16[:, c : c + 1],
                    rhs=ek[:],
                    start=(c == 0),
                    stop=(c == NCH - 1),
                )
            nc.vector.tensor_copy(kde_sb[0:1, sl], kp[:])
        if rep_ctx is not None:
            rep_ctx.__exit__(None, None, None)

        # ---- normalizer: allreduce the local sum, scale, store ----
        ssum = const.tile([1, 1], f32)
        nc.vector.tensor_reduce(
            ssum[:], kde_sb[:], axis=mybir.AxisListType.X, op=mybir.AluOpType.add
        )
        if collective:
            ccin = dram.tile([1, 1], f32)
            ccout = dram.tile([1, 1], f32)
            nc.sync.dma_start(out=ccin[:], in_=ssum[:])
            nc.gpsimd.collective_compute(
                "AllReduce",
                mybir.AluOpType.add,
                replica_groups=[list(range(NCORES))],
                ins=[ccin.opt()],
                outs=[ccout.opt()],
            )
            stot = const.tile([1, 1], f32)
            nc.sync.dma_start(out=stot[:], in_=ccout[:])
        else:
            stot = ssum
        rec = const.tile([1, 1], f32)
        nc.vector.reciprocal(rec[:], stot[:])
        kout = const.tile([1, ROWS], f32)
        nc.vector.tensor_scalar_mul(kout[:], kde_sb[:], rec[:])
        nc.sync.dma_start(out=out_d[:], in_=kout[:])

    nc.compile()
    return nc


def _get_program():
    global _PROGRAM
    if _PROGRAM is None:
        _PROGRAM = _build_program()
    return _PROGRAM


def _host_inputs(space_probs, cov_inv):
    """Build the per-core input maps (host-side layout/shard prep only)."""
    p = np.asarray(space_probs, dtype=np.float32).reshape(-1)
    a = np.asarray(cov_inv, dtype=np.float32)

    idx = np.indices(GRID, dtype=np.float32).reshape(3, N)  # [3, N], i fastest-major
    c = idx - 9.5  # centered; values +-0.5..+-9.5 are exact in fp16

    lc = np.zeros((8, NPAD), dtype=np.float16)
    lc[0:3, :N] = c
    lc[3, :N] = 1.0
    lc[4:7, :N] = c
    lc[7, :N] = 1.0

    csq = np.zeros((9, NPAD), dtype=np.float32)
    k = 0
    for ai in range(3):
        for bi in range(3):
            csq[k, :N] = c[ai] * c[bi]
            k += 1

    pcol = np.zeros((NCH, 128), dtype=np.float32)
    pcol.reshape(-1)[:N] = p
    pcol = np.ascontiguousarray(pcol.T)  # [128, NCH]

    cov9 = np.ascontiguousarray(a.reshape(9, 1))
    cov3 = np.ascontiguousarray(a)

    in_maps = []
    for r in range(NCORES):
        sl = slice(r * ROWS, (r + 1) * ROWS)
        in_maps.append(
            {
                "lc": lc,
                "csq": csq,
                "cov9": cov9,
                "cov3": cov3,
                "ctl": np.ascontiguousarray(c[:, sl]),
                "csql": np.ascontiguousarray(csq[:, sl]),
                "pcol": pcol,
            }
        )
    return in_maps


def kernel(space_probs, cov_inv):
    from concourse.bass_utils import run_bass_kernel_spmd

    nc = _get_program()
    in_maps = _host_inputs(space_probs, cov_inv)
    res = run_bass_kernel_spmd(nc, in_maps, list(range(NCORES)))
    out = np.concatenate(
        [res.results[r]["out"].reshape(-1) for r in range(NCORES)]
    )
    return out.reshape(GRID).astype(np.float32)



# revision 12
# speedup vs baseline: 6.2344x; 6.2344x over previous
"""KDE on a 20^3 grid, distributed across 8 TRN2 NeuronCores.

Separable-factorization algorithm (replaces the dense 8000x8000 kernel
matrix): with A = cov_inv, q_v = v^T A v, and centered grid coords,

  kde[i] = sum_b p_b exp(-0.5(q_i + q_b - 2 GA_i . x_b))
         = e^{F_i} * sum_{x1,x2,x3} w'[x1,x2,x3] E1[x1,i] E2[x2,i] E3[x3,i]

where GA_i . x_b = sum_k g_k,i x_k,b factorizes over the tensor-product
grid.  Per-axis tables absorb t_k(x) = 0.5 A_kk x^2 (so the b-side factor
w' = p * exp(-(cross terms)) stays inside fp32 range) and per-i shifts
s_k,i = max(0, 9.5|g_k,i| - 30) (so E-table entries and partial sums
stay inside fp32 range); F_i = -0.5 q_i + sum_k s_k,i compensates.

Device work per core (1000 query rows i, full b-grid):
  - build E tables on-chip: tiny matmuls form the exponent tiles in
    PSUM, ScalarE exponentiates (E3 in bf16 hi/lo for the PE matmul).
  - w' = p (.) Wfac in one vector op, cast bf16.
  - stage 1 (PE): out1[i, x12] = sum_x3 E3[x3,i] w'[x3,x12], 8 i-chunks
    of 128, two accumulating bf16 matmuls (hi/lo) each.
  - stage 2 (DVE): multiply by E2 (broadcast AP) + grouped reduce x2.
  - stage 3 (DVE): tensor_tensor_reduce with E1 -> kde column.
  - final: multiply by e^{F}, 32x32 block-transpose, one DMA out.
Normalization (a global scalar) happens on the host after gathering.
"""

import numpy as np

GRID = (20, 20, 20)
N = 8000
NCORES = 8
ROWS = N // NCORES          # 1000 query rows per core
NCH = 8                     # i-chunks of 128 (last 24 cols are padding)
NI = NCH * 128              # 1024 padded rows per core
SHIFT_B = 30.0              # per-axis shift budget

_PROGRAM = None


def _build_program(num_devices=NCORES, stage=4):
    from contextlib import ExitStack

    import concourse.bacc as bacc
    import concourse.mybir as mybir
    import concourse.tile as tile

    f32 = mybir.dt.float32
    bf16 = mybir.dt.bfloat16
    AX = mybir.AxisListType
    OP = mybir.AluOpType
    EXP = mybir.ActivationFunctionType.Exp

    nc = bacc.Bacc(
        "TRN2",
        target_bir_lowering=False,
        debug=False,
        num_devices=num_devices,
    )

    pt_d = nc.dram_tensor("pt", [20, 400], f32, kind="ExternalInput").ap()
    wfac_d = nc.dram_tensor("wfac", [20, 400], f32, kind="ExternalInput").ap()
    gscol_d = nc.dram_tensor("gscol", [5, NI], f32, kind="ExternalInput").ap()
    g3row_d = nc.dram_tensor("g3row", [3, NI], f32, kind="ExternalInput").ap()
    fmat_d = nc.dram_tensor("fmat", [128, NCH], f32, kind="ExternalInput").ap()
    etpat_d = nc.dram_tensor("etpat", [5, 40], f32, kind="ExternalInput").ap()
    e3pat_d = nc.dram_tensor("e3pat", [3, 20], f32, kind="ExternalInput").ap()
    out_d = nc.dram_tensor("out", [8, 128], f32, kind="ExternalOutput").ap()

    with tile.TileContext(nc) as tc, ExitStack() as ctx:
        from contextlib import ExitStack as _ES

        const = ctx.enter_context(tc.tile_pool(name="const", bufs=1))
        work = ctx.enter_context(tc.tile_pool(name="work", bufs=3))
        pre_ctx = _ES()
        psum_pre = pre_ctx.enter_context(
            tc.tile_pool(name="psum_pre", bufs=1, space="PSUM")
        )

        # ---- input loads ----
        pt_sb = const.tile([20, 400], f32)
        nc.sync.dma_start(out=pt_sb[:], in_=pt_d[:])
        wfac_sb = const.tile([20, 400], f32)
        nc.sync.dma_start(out=wfac_sb[:], in_=wfac_d[:])
        gscol_sb = const.tile([5, NI], f32)
        nc.sync.dma_start(out=gscol_sb[:], in_=gscol_d[:])
        g3row_sb = const.tile([3, NI], f32)
        nc.sync.dma_start(out=g3row_sb[:], in_=g3row_d[:])
        fmat_sb = const.tile([128, NCH], f32)
        nc.sync.dma_start(out=fmat_sb[:], in_=fmat_d[:])
        etpat_sb = const.tile([5, 40], f32)
        nc.sync.dma_start(out=etpat_sb[:], in_=etpat_d[:])
        e3pat_sb = const.tile([3, 20], f32)
        nc.sync.dma_start(out=e3pat_sb[:], in_=e3pat_d[:])

        # ---- w' = p * Wfac, cast to bf16 (one fused vector op) ----
        whi = const.tile([20, 400], bf16)
        nc.vector.scalar_tensor_tensor(
            whi[:], pt_sb[:], 1.0, wfac_sb[:], op0=OP.mult, op1=OP.mult
        )

        if stage == 0:
            # smoke test: copy a slice of an input to the output
            kout0 = const.tile([32, 128], f32)
            nc.vector.memset(kout0[:], 0.0)
            nc.vector.tensor_copy(kout0[0:5, 0:128], gscol_sb[:, 0:128])
            nc.sync.dma_start(out=out_d[:], in_=kout0[0:8, :])

        e3hi = e3lo = et = ef = kdeT = None
        if stage >= 1:
            # ---- E3 [x3=20, i=1024] in bf16 hi/lo: PE exponent + ScalarE exp
            e3hi = const.tile([20, NI], bf16)
            e3lo = const.tile([20, NI], bf16)
            for h in range(2):
                sl = slice(h * 512, (h + 1) * 512)
                xp3 = psum_pre.tile([20, 512], f32, tag="xp3", bufs=2)
                nc.tensor.matmul(
                    xp3[:], lhsT=e3pat_sb[:], rhs=g3row_sb[:, sl], start=True, stop=True
                )
                nc.scalar.activation(e3hi[:, sl], xp3[:], EXP)
                xf3 = psum_pre.tile([20, 512], f32, tag="xf3", bufs=2)
                nc.scalar.activation(xf3[:], xp3[:], EXP)
                nc.vector.tensor_sub(e3lo[:, sl], xf3[:], e3hi[:, sl])

            # ---- E1/E2 [i-chunk 128, 20] fp32, packed [128, 8*40] ----
            xpe = psum_pre.tile([128, NCH * 40], f32)
            for ci in range(NCH):
                nc.tensor.matmul(
                    xpe[:, ci * 40 : ci * 40 + 40],
                    lhsT=gscol_sb[:, ci * 128 : (ci + 1) * 128],
                    rhs=etpat_sb[:],
                    start=True,
                    stop=True,
                )
            et = const.tile([128, NCH * 40], f32)
            nc.scalar.activation(et[:], xpe[:], EXP)

            # ---- e^{F} ----
            ef = const.tile([128, NCH], f32)
            nc.scalar.activation(ef[:], fmat_sb[:], EXP)

            if stage == 1:
                # E-tables only: dump a slice of et
                kout1 = const.tile([32, 128], f32)
                nc.vector.memset(kout1[:], 0.0)
                nc.vector.tensor_copy(kout1[0:8, :], et[0:8, 0:128])
                nc.sync.dma_start(out=out_d[:], in_=kout1[0:8, :])

        pre_ctx.close()

        if stage >= 2:
            psum_main = ctx.enter_context(
                tc.tile_pool(name="psum_main", bufs=3, space="PSUM")
            )
            sub = stage - 20 if 20 <= stage < 24 else 3

            # ---- main loop over 8 i-chunks ----
            kdeT = const.tile([128, NCH], f32)
            nc.vector.memset(kdeT[:], 0.0)
            for ci in range(NCH):
                isl = slice(ci * 128, (ci + 1) * 128)
                o1p = psum_main.tile([128, 400], f32)
                nc.tensor.matmul(
                    o1p[:], lhsT=e3hi[:, isl], rhs=whi[:], start=True, stop=False
                )
                nc.tensor.matmul(
                    o1p[:], lhsT=e3lo[:, isl], rhs=whi[:], start=False, stop=True
                )
                if sub == 0:
                    nc.vector.tensor_copy(kdeT[:, ci : ci + 1], o1p[:, 0:1])
                    continue
                tmp = work.tile([128, 400], f32, tag="tmp")
                e2b = (
                    et[:, ci * 40 + 20 : ci * 40 + 40]
                    .unsqueeze(1)
                    .broadcast_to((128, 20, 20))
                )
                o1p3 = o1p[:].rearrange("p (a b) -> p a b", a=20, b=20)
                tmp3 = tmp[:].rearrange("p (a b) -> p a b", a=20, b=20)
                nc.vector.tensor_mul(tmp3, o1p3, e2b)
                if sub == 1:
                    nc.vector.tensor_copy(kdeT[:, ci : ci + 1], tmp[:, 0:1])
                    continue
                out2 = work.tile([128, 20], f32, tag="out2")
                nc.vector.tensor_reduce(out2[:], tmp3, axis=AX.X, op=OP.add)
                if sub == 2:
                    nc.vector.tensor_copy(kdeT[:, ci : ci + 1], out2[:, 0:1])
                    continue
                out3 = work.tile([128, 20], f32, tag="out3")
                nc.vector.tensor_mul(out3[:], out2[:], et[:, ci * 40 : ci * 40 + 20])
                nc.vector.tensor_reduce(
                    kdeT[:, ci : ci + 1], out3[:], axis=AX.X, op=OP.add
                )

            if stage == 2 or 20 <= stage < 24:
                # main loop done: dump kdeT columns without transpose
                kout2 = const.tile([32, 128], f32)
                nc.vector.memset(kout2[:], 0.0)
                nc.vector.tensor_copy(kout2[0:8, 0:NCH], kdeT[0:8, :])
                nc.sync.dma_start(out=out_d[:], in_=kout2[0:8, :])

        if stage >= 3:
            # ---- scale by e^{F}, transpose to row-major i order, DMA out ----
            ksc = const.tile([128, 32], f32)
            nc.vector.memset(ksc[:], 0.0)
            nc.vector.tensor_mul(ksc[:, 0:NCH], kdeT[:], ef[:])
            t32 = const.tile([128, 32], f32)
            nc.vector.transpose(t32[:], ksc[:])
            kout = const.tile([32, 128], f32)
            for r in range(4):
                nc.vector.tensor_copy(
                    kout[0:8, r * 32 : (r + 1) * 32], t32[r * 32 : r * 32 + 8, 0:32]
                )
            nc.sync.dma_start(out=out_d[:], in_=kout[0:8, :])

    nc.compile()
    return nc


def _get_program():
    global _PROGRAM
    if _PROGRAM is None:
        _PROGRAM = _build_program()
    return _PROGRAM


def _host_inputs(space_probs, cov_inv):
    """Per-core input maps: host-side layout + coordinate-table prep."""
    p = np.asarray(space_probs, dtype=np.float64).reshape(-1)
    A = np.asarray(cov_inv, dtype=np.float64)

    idx = np.indices(GRID, dtype=np.float64).reshape(3, N)
    cc = idx - 9.5                        # centered coords, [3, N]
    c20 = np.arange(20, dtype=np.float64) - 9.5

    G = (cc.T @ A).T                      # [3, N] g_k,i
    q = np.sum(cc * G, axis=0)            # [N]
    s = np.maximum(0.0, 9.5 * np.abs(G) - SHIFT_B)   # [3, N]
    F = -0.5 * q + s.sum(axis=0)          # [N]
    t = 0.5 * np.diag(A)[:, None] * (c20**2)[None, :]  # [3, 20]

    crossexp = -(
        A[0, 1] * cc[0] * cc[1] + A[0, 2] * cc[0] * cc[2] + A[1, 2] * cc[1] * cc[2]
    )
    wfac = np.exp(crossexp).astype(np.float32).reshape(400, 20).T
    pt = p.astype(np.float32).reshape(400, 20).T

    # etpat rows match gscol rows [g1, g2, s1, s2, ones]
    etpat = np.zeros((5, 40), dtype=np.float32)
    etpat[0, 0:20] = c20
    etpat[2, 0:20] = -1.0
    etpat[4, 0:20] = -t[0]
    etpat[1, 20:40] = c20
    etpat[3, 20:40] = -1.0
    etpat[4, 20:40] = -t[1]

    # e3pat rows match g3row rows [g3, ones, s3]
    e3pat = np.zeros((3, 20), dtype=np.float32)
    e3pat[0] = c20
    e3pat[1] = -t[2]
    e3pat[2] = -1.0

    in_maps = []
    for r in range(NCORES):
        i0 = r * ROWS
        sl = slice(i0, i0 + ROWS)
        gscol = np.zeros((5, NI), dtype=np.float32)
        gscol[0, :ROWS] = G[0, sl]
        gscol[1, :ROWS] = G[1, sl]
        gscol[2, :ROWS] = s[0, sl]
        gscol[3, :ROWS] = s[1, sl]
        gscol[4, :ROWS] = 1.0
        g3row = np.zeros((3, NI), dtype=np.float32)
        g3row[0, :ROWS] = G[2, sl]
        g3row[1, :ROWS] = 1.0
        g3row[2, :ROWS] = s[2, sl]
        fm = np.zeros((NCH, 128), dtype=np.float32)
        fm.reshape(-1)[:ROWS] = F[sl]
        fmat = fm.T
        in_maps.append(
            {
                "pt": pt,
                "wfac": wfac,
                "gscol": gscol,
                "g3row": g3row,
                "fmat": np.ascontiguousarray(fmat),
                "etpat": etpat,
                "e3pat": e3pat,
            }
        )
    return in_maps


def kernel(space_probs, cov_inv):
    from concourse.bass_utils import run_bass_kernel_spmd

    nc = _get_program()
    in_maps = _host_inputs(space_probs, cov_inv)
    res = run_bass_kernel_spmd(nc, in_maps, list(range(NCORES)))
    out = np.concatenate(
        [res.results[r]["out"].reshape(-1)[:ROWS] for r in range(NCORES)]
    )
    out = out / out.sum(dtype=np.float64)
    return out.reshape(GRID).astype(np.float32)


# revision 13
# speedup vs baseline: 7.7822x; 1.2483x over previous
"""KDE on a 20^3 grid, distributed across 8 TRN2 NeuronCores.

Separable-factorization algorithm (replaces the dense 8000x8000 kernel
matrix): with A = cov_inv, q_v = v^T A v, and centered grid coords,

  kde[i] = sum_b p_b exp(-0.5(q_i + q_b - 2 GA_i . x_b))
         = e^{F_i} * sum_{x1,x2,x3} w'[x1,x2,x3] E1[x1,i] E2[x2,i] E3[x3,i]

since GA_i . x_b = sum_k g_k,i x_k,b factorizes over the tensor-product
grid.  Per-axis tables absorb t_k(x) = 0.5 A_kk x^2 (so the b-side
factor w' = p * exp(-(cross terms)) stays inside fp32 range) and per-i
shifts s_k,i = max(0, 9.5|g_k,i| - 30) (so E-table entries and partial
sums stay inside fp32 range); F_i = -0.5 q_i + sum_k s_k,i compensates.

Device pipeline per core (1000 query rows i, full b-grid, 8 i-chunks):
  - 3 packed input DMAs; exponent tiles built by tiny fp16 hi/lo
    matmuls (1-pass PE), ScalarE exponentiates.
  - E3 hi/lo stacked to k=40 so stage 1 is ONE bf16 matmul per chunk:
    out1[i, x12] = sum_x3 E3[x3,i] w'[x3,x12] in PSUM.
  - E12 = E1 (x) E2 prebuilt per chunk on DVE during the preamble.
  - main loop: PE matmul -> ScalarE PSUM->bf16 copy -> DVE multiply by
    E12 -> DVE XY-reduce into the kde column (3 engines pipelined).
  - final: multiply by e^{F}, 32x32 block-transpose, one DMA out.
Normalization (a global scalar) happens on the host after gathering.
"""

import numpy as np

GRID = (20, 20, 20)
N = 8000
NCORES = 8
ROWS = N // NCORES          # 1000 query rows per core
NCH = 8                     # i-chunks of 128 (last 24 cols are padding)
NI = NCH * 128              # 1024 padded rows per core
SHIFT_B = 30.0              # per-axis shift budget

# pats (fp16) column layout
_GS0 = 0                    # gscol10 [10, 1024]
_G3 = 1024                  # g3row6  [6, 1024]
_ETP = 2048                 # etpat10 [10, 40]
_E3P = 2088                 # e3pat6  [6, 20]
_PATW = 2108

_PROGRAM = None


def _build_program(num_devices=NCORES):
    from contextlib import ExitStack

    import concourse.bacc as bacc
    import concourse.mybir as mybir
    import concourse.tile as tile

    f32 = mybir.dt.float32
    f16 = mybir.dt.float16
    bf16 = mybir.dt.bfloat16
    AX = mybir.AxisListType
    OP = mybir.AluOpType
    EXP = mybir.ActivationFunctionType.Exp
    CPY = mybir.ActivationFunctionType.Copy

    nc = bacc.Bacc(
        "TRN2",
        target_bir_lowering=False,
        debug=False,
        num_devices=num_devices,
    )

    pw_d = nc.dram_tensor("pw", [20, 800], f32, kind="ExternalInput").ap()
    pats_d = nc.dram_tensor("pats", [10, _PATW], f16, kind="ExternalInput").ap()
    fmat_d = nc.dram_tensor("fmat", [128, NCH], f32, kind="ExternalInput").ap()
    out_d = nc.dram_tensor("out", [8, 128], f32, kind="ExternalOutput").ap()

    with tile.TileContext(nc) as tc, ExitStack() as ctx:
        from contextlib import ExitStack as _ES

        const = ctx.enter_context(tc.tile_pool(name="const", bufs=1))
        work = ctx.enter_context(tc.tile_pool(name="work", bufs=3))
        pre_ctx = _ES()
        psum_pre = pre_ctx.enter_context(
            tc.tile_pool(name="psum_pre", bufs=1, space="PSUM")
        )

        # ---- input loads (3 packed DMAs) ----
        pw_sb = const.tile([20, 800], f32)
        nc.sync.dma_start(out=pw_sb[:], in_=pw_d[:])
        pats_sb = const.tile([10, _PATW], f16)
        nc.sync.dma_start(out=pats_sb[:], in_=pats_d[:])
        fmat_sb = const.tile([128, NCH], f32)
        nc.sync.dma_start(out=fmat_sb[:], in_=fmat_d[:])

        # ---- w' = p * Wfac -> bf16, duplicated to rows 20-39 for k=40 ----
        whi40 = const.tile([40, 400], bf16)
        nc.vector.scalar_tensor_tensor(
            whi40[0:20, :], pw_sb[:, 0:400], 1.0, pw_sb[:, 400:800],
            op0=OP.mult, op1=OP.mult,
        )
        nc.sync.dma_start(out=whi40[20:40, :], in_=whi40[0:20, :])

        # ---- E3 [x3, i] exponents: fp16 k=6 matmuls, hi/lo stacked k=40 ----
        e3s = const.tile([40, NI], bf16)
        e3lo_st = const.tile([20, NI], bf16)
        for h in range(2):
            sl = slice(h * 512, (h + 1) * 512)
            xp3 = psum_pre.tile([20, 512], f32, tag="xp3", bufs=2)
            nc.tensor.matmul(
                xp3[:],
                lhsT=pats_sb[0:6, _E3P : _E3P + 20],
                rhs=pats_sb[0:6, _G3 + h * 512 : _G3 + h * 512 + 512],
                start=True,
                stop=True,
            )
            nc.scalar.activation(e3s[0:20, sl], xp3[:], EXP)
            xf3 = psum_pre.tile([20, 512], f32, tag="xf3", bufs=2)
            nc.scalar.activation(xf3[:], xp3[:], EXP)
            nc.vector.tensor_sub(e3lo_st[:, sl], xf3[:], e3s[0:20, sl])
        nc.sync.dma_start(out=e3s[20:40, :], in_=e3lo_st[:])

        # ---- E1/E2 exponents: fp16 k=10 matmuls, packed [128, 8*40] ----
        xpe = psum_pre.tile([128, NCH * 40], f32)
        for ci in range(NCH):
            nc.tensor.matmul(
                xpe[:, ci * 40 : ci * 40 + 40],
                lhsT=pats_sb[:, ci * 128 : (ci + 1) * 128],
                rhs=pats_sb[:, _ETP : _ETP + 40],
                start=True,
                stop=True,
            )
        et = const.tile([128, NCH * 40], f32)
        nc.scalar.activation(et[:], xpe[:], EXP)

        # ---- e^{F} ----
        ef = const.tile([128, NCH], f32)
        nc.scalar.activation(ef[:], fmat_sb[:], EXP)

        # ---- E12 = E1 (x) E2, bf16, per chunk [128, 400] ----
        e12 = const.tile([128, NCH * 400], bf16)
        for ci in range(NCH):
            e1b = (
                et[:, ci * 40 : ci * 40 + 20]
                .unsqueeze(2)
                .broadcast_to((128, 20, 20))
            )
            e2b = (
                et[:, ci * 40 + 20 : ci * 40 + 40]
                .unsqueeze(1)
                .broadcast_to((128, 20, 20))
            )
            e12v = e12[:, ci * 400 : ci * 400 + 400].rearrange(
                "p (a b) -> p a b", a=20, b=20
            )
            nc.vector.tensor_mul(e12v, e1b, e2b)

        pre_ctx.close()
        psum_main = ctx.enter_context(
            tc.tile_pool(name="psum_main", bufs=3, space="PSUM")
        )

        # ---- main loop over 8 i-chunks ----
        kdeT = const.tile([128, NCH], f32)
        for ci in range(NCH):
            isl = slice(ci * 128, (ci + 1) * 128)
            o1p = psum_main.tile([128, 400], f32)
            nc.tensor.matmul(
                o1p[:], lhsT=e3s[:, isl], rhs=whi40[:], start=True, stop=True
            )
            tmpb = work.tile([128, 400], bf16, tag="tmpb")
            nc.scalar.activation(tmpb[:], o1p[:], CPY)
            prod = work.tile([128, 400], bf16, tag="prod")
            prod3 = prod[:].rearrange("p (a b) -> p a b", a=20, b=20)
            tmpb3 = tmpb[:].rearrange("p (a b) -> p a b", a=20, b=20)
            e12v = e12[:, ci * 400 : ci * 400 + 400].rearrange(
                "p (a b) -> p a b", a=20, b=20
            )
            nc.vector.tensor_mul(prod3, tmpb3, e12v)
            nc.vector.tensor_reduce(
                kdeT[:, ci : ci + 1], prod3, axis=AX.XY, op=OP.add
            )

        # ---- scale by e^{F}, transpose to row-major i order, DMA out ----
        ksc = const.tile([128, 32], f32)
        nc.vector.memset(ksc[:], 0.0)
        nc.vector.tensor_mul(ksc[:, 0:NCH], kdeT[:], ef[:])
        t32 = const.tile([128, 32], f32)
        nc.vector.transpose(t32[:], ksc[:])
        kout = const.tile([32, 128], f32)
        for r in range(4):
            nc.vector.tensor_copy(
                kout[0:8, r * 32 : (r + 1) * 32], t32[r * 32 : r * 32 + 8, 0:32]
            )
        nc.sync.dma_start(out=out_d[:], in_=kout[0:8, :])

    nc.compile()
    return nc


def _get_program():
    global _PROGRAM
    if _PROGRAM is None:
        _PROGRAM = _build_program()
    return _PROGRAM


def _split16(v):
    hi = v.astype(np.float16).astype(np.float64)
    return hi, v - hi


def _host_inputs(space_probs, cov_inv):
    """Per-core input maps: host-side layout + coordinate-table prep."""
    p = np.asarray(space_probs, dtype=np.float64).reshape(-1)
    A = np.asarray(cov_inv, dtype=np.float64)

    idx = np.indices(GRID, dtype=np.float64).reshape(3, N)
    cc = idx - 9.5                        # centered coords, [3, N]
    c20 = np.arange(20, dtype=np.float64) - 9.5

    G = (cc.T @ A).T                      # [3, N] g_k,i
    q = np.sum(cc * G, axis=0)            # [N]
    s = np.maximum(0.0, 9.5 * np.abs(G) - SHIFT_B)   # [3, N]
    F = -0.5 * q + s.sum(axis=0)          # [N]
    t = 0.5 * np.diag(A)[:, None] * (c20**2)[None, :]  # [3, 20]

    crossexp = -(
        A[0, 1] * cc[0] * cc[1] + A[0, 2] * cc[0] * cc[2] + A[1, 2] * cc[1] * cc[2]
    )
    wfac = np.exp(crossexp).reshape(400, 20).T
    pt = p.reshape(400, 20).T
    pw = np.zeros((20, 800), dtype=np.float32)
    pw[:, 0:400] = pt
    pw[:, 400:800] = wfac

    th = [_split16(t[k]) for k in range(3)]

    # etpat10 rows: [g1h,g1l,g2h,g2l,s1h,s1l,s2h,s2l,1(t hi),1(t lo)]
    etpat = np.zeros((10, 40), dtype=np.float16)
    etpat[0, 0:20] = c20
    etpat[1, 0:20] = c20
    etpat[4, 0:20] = -1.0
    etpat[5, 0:20] = -1.0
    etpat[8, 0:20] = -th[0][0]
    etpat[9, 0:20] = -th[0][1]
    etpat[2, 20:40] = c20
    etpat[3, 20:40] = c20
    etpat[6, 20:40] = -1.0
    etpat[7, 20:40] = -1.0
    etpat[8, 20:40] = -th[1][0]
    etpat[9, 20:40] = -th[1][1]

    # e3pat6 rows: [g3h, g3l, s3h, s3l, 1(t hi), 1(t lo)]
    e3pat = np.zeros((6, 20), dtype=np.float16)
    e3pat[0] = c20
    e3pat[1] = c20
    e3pat[2] = -1.0
    e3pat[3] = -1.0
    e3pat[4] = -th[2][0]
    e3pat[5] = -th[2][1]

    in_maps = []
    for r in range(NCORES):
        i0 = r * ROWS
        sl = slice(i0, i0 + ROWS)

        pats = np.zeros((10, _PATW), dtype=np.float16)
        hi_rows = (0, 2, 4, 6)
        lo_rows = (1, 3, 5, 7)
        for k, src in enumerate((G[0], G[1], s[0], s[1])):
            hi, lo = _split16(src[sl])
            pats[hi_rows[k], _GS0 : _GS0 + ROWS] = hi
            pats[lo_rows[k], _GS0 : _GS0 + ROWS] = lo
        pats[8, _GS0 : _GS0 + ROWS] = 1.0
        pats[9, _GS0 : _GS0 + ROWS] = 1.0

        g3h, g3l = _split16(G[2][sl])
        s3h, s3l = _split16(s[2][sl])
        pats[0, _G3 : _G3 + ROWS] = g3h
        pats[1, _G3 : _G3 + ROWS] = g3l
        pats[2, _G3 : _G3 + ROWS] = s3h
        pats[3, _G3 : _G3 + ROWS] = s3l
        pats[4, _G3 : _G3 + ROWS] = 1.0
        pats[5, _G3 : _G3 + ROWS] = 1.0

        pats[:, _ETP : _ETP + 40] = etpat
        pats[0:6, _E3P : _E3P + 20] = e3pat

        fm = np.zeros((NCH, 128), dtype=np.float32)
        fm.reshape(-1)[:ROWS] = F[sl]
        fmat = np.ascontiguousarray(fm.T)

        in_maps.append({"pw": pw, "pats": pats, "fmat": fmat})
    return in_maps


def kernel(space_probs, cov_inv):
    from concourse.bass_utils import run_bass_kernel_spmd

    nc = _get_program()
    in_maps = _host_inputs(space_probs, cov_inv)
    res = run_bass_kernel_spmd(nc, in_maps, list(range(NCORES)))
    out = np.concatenate(
        [res.results[r]["out"].reshape(-1)[:ROWS] for r in range(NCORES)]
    )
    out = out / out.sum(dtype=np.float64)
    return out.reshape(GRID).astype(np.float32)


# revision 14
# speedup vs baseline: 8.6484x; 1.1113x over previous
"""KDE on a 20^3 grid, distributed across 8 TRN2 NeuronCores.

Separable-factorization algorithm (replaces the dense 8000x8000 kernel
matrix): with A = cov_inv, q_v = v^T A v, and centered grid coords,

  kde[i] = sum_b p_b exp(-0.5(q_i + q_b - 2 GA_i . x_b))
         = e^{F_i} * sum_{x1,x2,x3} w'[x1,x2,x3] E1[x1,i] E2[x2,i] E3[x3,i]

since GA_i . x_b = sum_k g_k,i x_k,b factorizes over the tensor-product
grid.  Per-axis tables absorb t_k(x) = 0.5 A_kk x^2 (so the b-side
factor w' = p * exp(-(cross terms)) stays inside fp32 range) and per-i
shifts s_k,i = max(0, 9.5|g_k,i| - 30) (so E-table entries and partial
sums stay inside fp32 range); F_i = -0.5 q_i + sum_k s_k,i compensates.

Device pipeline per core (1000 query rows i, full b-grid, 8 i-chunks):
  - 3 packed input DMAs; exponent tiles built by tiny fp16 hi/lo
    matmuls (1-pass PE), ScalarE exponentiates to bf16/f32.
  - stage 1 is ONE bf16 matmul per chunk (k=20):
    out1[i, x12] = sum_x3 E3[x3,i] w'[x3,x12] into PSUM.
  - E12 = E1 (x) E2 prebuilt per chunk on GpSimd (SBUF-only engine).
  - main loop: PE matmul -> one DVE scalar_tensor_tensor that reads
    PSUM, multiplies by E12, and free-dim-accumulates into the kde
    column (accum_out).
  - final: multiply by e^{F}, 32x32 block-transpose, one DMA out.
Normalization (a global scalar) happens on the host after gathering.
"""

import numpy as np

GRID = (20, 20, 20)
N = 8000
NCORES = 8
ROWS = N // NCORES          # 1000 query rows per core
NCH = 8                     # i-chunks of 128 (last 24 cols are padding)
NI = NCH * 128              # 1024 padded rows per core
SHIFT_B = 30.0              # per-axis shift budget

# pats (fp16) column layout
_GS0 = 0                    # gscol10 [10, 1024]
_G3 = 1024                  # g3row6  [6, 1024]
_ETP = 2048                 # etpat10 [10, 40]
_E3P = 2088                 # e3pat6  [6, 20]
_PATW = 2108

_PROGRAM = None


def _build_program(num_devices=NCORES):
    from contextlib import ExitStack

    import concourse.bacc as bacc
    import concourse.mybir as mybir
    import concourse.tile as tile

    f32 = mybir.dt.float32
    f16 = mybir.dt.float16
    bf16 = mybir.dt.bfloat16
    OP = mybir.AluOpType
    EXP = mybir.ActivationFunctionType.Exp

    nc = bacc.Bacc(
        "TRN2",
        target_bir_lowering=False,
        debug=False,
        num_devices=num_devices,
    )

    pw_d = nc.dram_tensor("pw", [20, 800], f32, kind="ExternalInput").ap()
    pats_d = nc.dram_tensor("pats", [10, _PATW], f16, kind="ExternalInput").ap()
    fmat_d = nc.dram_tensor("fmat", [128, NCH], f32, kind="ExternalInput").ap()
    out_d = nc.dram_tensor("out", [8, 128], f32, kind="ExternalOutput").ap()

    with tile.TileContext(nc) as tc, ExitStack() as ctx:
        from contextlib import ExitStack as _ES

        const = ctx.enter_context(tc.tile_pool(name="const", bufs=1))
        work = ctx.enter_context(tc.tile_pool(name="work", bufs=3))
        pre_ctx = _ES()
        psum_pre = pre_ctx.enter_context(
            tc.tile_pool(name="psum_pre", bufs=1, space="PSUM")
        )

        # ---- input loads (3 packed DMAs) ----
        pw_sb = const.tile([20, 800], f32)
        nc.sync.dma_start(out=pw_sb[:], in_=pw_d[:])
        pats_sb = const.tile([10, _PATW], f16)
        nc.sync.dma_start(out=pats_sb[:], in_=pats_d[:])
        fmat_sb = const.tile([128, NCH], f32)
        nc.sync.dma_start(out=fmat_sb[:], in_=fmat_d[:])

        # ---- w' = p * Wfac -> bf16 (one fused vector op) ----
        whi = const.tile([20, 400], bf16)
        nc.vector.scalar_tensor_tensor(
            whi[:], pw_sb[:, 0:400], 1.0, pw_sb[:, 400:800],
            op0=OP.mult, op1=OP.mult,
        )

        # ---- E3 [x3, i]: fp16 k=6 exponent matmuls -> Exp -> bf16 ----
        e3s = const.tile([20, NI], bf16)
        for h in range(2):
            sl = slice(h * 512, (h + 1) * 512)
            xp3 = psum_pre.tile([20, 512], f32, tag="xp3", bufs=2)
            nc.tensor.matmul(
                xp3[:],
                lhsT=pats_sb[0:6, _E3P : _E3P + 20],
                rhs=pats_sb[0:6, _G3 + h * 512 : _G3 + h * 512 + 512],
                start=True,
                stop=True,
            )
            nc.scalar.activation(e3s[:, sl], xp3[:], EXP)

        # ---- E1/E2 exponents: fp16 k=10 matmuls, packed [128, 8*40] ----
        xpe = psum_pre.tile([128, NCH * 40], f32)
        for ci in range(NCH):
            nc.tensor.matmul(
                xpe[:, ci * 40 : ci * 40 + 40],
                lhsT=pats_sb[:, ci * 128 : (ci + 1) * 128],
                rhs=pats_sb[:, _ETP : _ETP + 40],
                start=True,
                stop=True,
            )
        et = const.tile([128, NCH * 40], f32)
        nc.scalar.activation(et[:], xpe[:], EXP)

        # ---- e^{F} ----
        ef = const.tile([128, NCH], f32)
        nc.scalar.activation(ef[:], fmat_sb[:], EXP)

        # ---- E12 = E1 (x) E2, bf16, built on GpSimd (keeps DVE free) ----
        e12 = const.tile([128, NCH * 400], bf16)
        for ci in range(NCH):
            e1b = (
                et[:, ci * 40 : ci * 40 + 20]
                .unsqueeze(2)
                .broadcast_to((128, 20, 20))
            )
            e2b = (
                et[:, ci * 40 + 20 : ci * 40 + 40]
                .unsqueeze(1)
                .broadcast_to((128, 20, 20))
            )
            e12v = e12[:, ci * 400 : ci * 400 + 400].rearrange(
                "p (a b) -> p a b", a=20, b=20
            )
            nc.gpsimd.tensor_mul(e12v, e1b, e2b)

        pre_ctx.close()
        psum_main = ctx.enter_context(
            tc.tile_pool(name="psum_main", bufs=4, space="PSUM")
        )

        # ---- main loop over 8 i-chunks ----
        kdeT = const.tile([128, NCH], f32)
        for ci in range(NCH):
            isl = slice(ci * 128, (ci + 1) * 128)
            o1p = psum_main.tile([128, 400], f32)
            nc.tensor.matmul(
                o1p[:], lhsT=e3s[:, isl], rhs=whi[:], start=True, stop=True
            )
            prod = work.tile([128, 400], bf16, tag="prod")
            nc.vector.scalar_tensor_tensor(
                prod[:],
                o1p[:],
                1.0,
                e12[:, ci * 400 : ci * 400 + 400],
                op0=OP.mult,
                op1=OP.mult,
                accum_out=kdeT[:, ci : ci + 1],
            )

        # ---- scale by e^{F}, transpose to row-major i order, DMA out ----
        ksc = const.tile([128, 32], f32)
        nc.vector.memset(ksc[:], 0.0)
        nc.vector.tensor_mul(ksc[:, 0:NCH], kdeT[:], ef[:])
        t32 = const.tile([128, 32], f32)
        nc.vector.transpose(t32[:], ksc[:])
        kout = const.tile([32, 128], f32)
        for r in range(4):
            nc.vector.tensor_copy(
                kout[0:8, r * 32 : (r + 1) * 32], t32[r * 32 : r * 32 + 8, 0:32]
            )
        nc.sync.dma_start(out=out_d[:], in_=kout[0:8, :])

    nc.compile()
    return nc


def _get_program():
    global _PROGRAM
    if _PROGRAM is None:
        _PROGRAM = _build_program()
    return _PROGRAM


def _split16(v):
    hi = v.astype(np.float16).astype(np.float64)
    return hi, v - hi


def _host_inputs(space_probs, cov_inv):
    """Per-core input maps: host-side layout + coordinate-table prep."""
    p = np.asarray(space_probs, dtype=np.float64).reshape(-1)
    A = np.asarray(cov_inv, dtype=np.float64)

    idx = np.indices(GRID, dtype=np.float64).reshape(3, N)
    cc = idx - 9.5                        # centered coords, [3, N]
    c20 = np.arange(20, dtype=np.float64) - 9.5

    G = (cc.T @ A).T                      # [3, N] g_k,i
    q = np.sum(cc * G, axis=0)            # [N]
    s = np.maximum(0.0, 9.5 * np.abs(G) - SHIFT_B)   # [3, N]
    F = -0.5 * q + s.sum(axis=0)          # [N]
    t = 0.5 * np.diag(A)[:, None] * (c20**2)[None, :]  # [3, 20]

    crossexp = -(
        A[0, 1] * cc[0] * cc[1] + A[0, 2] * cc[0] * cc[2] + A[1, 2] * cc[1] * cc[2]
    )
    wfac = np.exp(crossexp).reshape(400, 20).T
    pt = p.reshape(400, 20).T
    pw = np.zeros((20, 800), dtype=np.float32)
    pw[:, 0:400] = pt
    pw[:, 400:800] = wfac

    th = [_split16(t[k]) for k in range(3)]

    # etpat10 rows: [g1h,g1l,g2h,g2l,s1h,s1l,s2h,s2l,1(t hi),1(t lo)]
    etpat = np.zeros((10, 40), dtype=np.float16)
    etpat[0, 0:20] = c20
    etpat[1, 0:20] = c20
    etpat[4, 0:20] = -1.0
    etpat[5, 0:20] = -1.0
    etpat[8, 0:20] = -th[0][0]
    etpat[9, 0:20] = -th[0][1]
    etpat[2, 20:40] = c20
    etpat[3, 20:40] = c20
    etpat[6, 20:40] = -1.0
    etpat[7, 20:40] = -1.0
    etpat[8, 20:40] = -th[1][0]
    etpat[9, 20:40] = -th[1][1]

    # e3pat6 rows: [g3h, g3l, s3h, s3l, 1(t hi), 1(t lo)]
    e3pat = np.zeros((6, 20), dtype=np.float16)
    e3pat[0] = c20
    e3pat[1] = c20
    e3pat[2] = -1.0
    e3pat[3] = -1.0
    e3pat[4] = -th[2][0]
    e3pat[5] = -th[2][1]

    in_maps = []
    for r in range(NCORES):
        i0 = r * ROWS
        sl = slice(i0, i0 + ROWS)

        pats = np.zeros((10, _PATW), dtype=np.float16)
        hi_rows = (0, 2, 4, 6)
        lo_rows = (1, 3, 5, 7)
        for k, src in enumerate((G[0], G[1], s[0], s[1])):
            hi, lo = _split16(src[sl])
            pats[hi_rows[k], _GS0 : _GS0 + ROWS] = hi
            pats[lo_rows[k], _GS0 : _GS0 + ROWS] = lo
        pats[8, _GS0 : _GS0 + ROWS] = 1.0
        pats[9, _GS0 : _GS0 + ROWS] = 1.0

        g3h, g3l = _split16(G[2][sl])
        s3h, s3l = _split16(s[2][sl])
        pats[0, _G3 : _G3 + ROWS] = g3h
        pats[1, _G3 : _G3 + ROWS] = g3l
        pats[2, _G3 : _G3 + ROWS] = s3h
        pats[3, _G3 : _G3 + ROWS] = s3l
        pats[4, _G3 : _G3 + ROWS] = 1.0
        pats[5, _G3 : _G3 + ROWS] = 1.0

        pats[:, _ETP : _ETP + 40] = etpat
        pats[0:6, _E3P : _E3P + 20] = e3pat

        fm = np.zeros((NCH, 128), dtype=np.float32)
        fm.reshape(-1)[:ROWS] = F[sl]
        fmat = np.ascontiguousarray(fm.T)

        in_maps.append({"pw": pw, "pats": pats, "fmat": fmat})
    return in_maps


def kernel(space_probs, cov_inv):
    from concourse.bass_utils import run_bass_kernel_spmd

    nc = _get_program()
    in_maps = _host_inputs(space_probs, cov_inv)
    res = run_bass_kernel_spmd(nc, in_maps, list(range(NCORES)))
    out = np.concatenate(
        [res.results[r]["out"].reshape(-1)[:ROWS] for r in range(NCORES)]
    )
    out = out / out.sum(dtype=np.float64)
    return out.reshape(GRID).astype(np.float32)


# revision 15
# speedup vs baseline: 10.1108x; 1.1691x over previous
"""KDE on a 20^3 grid, distributed across 8 TRN2 NeuronCores.

Separable-factorization algorithm (replaces the dense 8000x8000 kernel
matrix): with A = cov_inv, q_v = v^T A v, and centered grid coords,

  kde[i] = sum_b p_b exp(-0.5(q_i + q_b - 2 GA_i . x_b))
         = e^{F_i} * sum_{x1,x2,x3} w'[x1,x2,x3] E1[x1,i] E2[x2,i] E3[x3,i]

since GA_i . x_b = sum_k g_k,i x_k,b factorizes over the tensor-product
grid.  Per-axis tables absorb t_k(x) = 0.5 A_kk x^2 (so the b-side
factor w' = p * exp(-(cross terms)) stays inside fp32 range) and per-i
shifts s_k,i = max(0, 9.5|g_k,i| - 30) (so E-table entries and partial
sums stay inside fp32 range); F_i = -0.5 q_i + sum_k s_k,i compensates.

Device pipeline per core (1000 query rows i, full b-grid, 8 i-chunks):
  - 3 packed input DMAs; exponent tiles built by tiny fp16 hi/lo
    matmuls (1-pass PE), ScalarE exponentiates to bf16/f32.
  - stage 1 is ONE bf16 matmul per chunk (k=20):
    out1[i, x12] = sum_x3 E3[x3,i] w'[x3,x12] into PSUM.
  - E12 = E1 (x) E2 prebuilt per chunk on GpSimd (SBUF-only engine).
  - main loop: PE matmul -> one DVE scalar_tensor_tensor that reads
    PSUM, multiplies by E12, and free-dim-accumulates into the kde
    column (accum_out).
  - final: multiply by e^{F}, 32x32 block-transpose, one DMA out.
Normalization (a global scalar) happens on the host after gathering.
"""

import numpy as np

GRID = (20, 20, 20)
N = 8000
NCORES = 8
ROWS = N // NCORES          # 1000 query rows per core
NCH = 8                     # i-chunks of 128 (last 24 cols are padding)
NI = NCH * 128              # 1024 padded rows per core
SHIFT_B = 30.0              # per-axis shift budget

# pats (fp16) column layout
_GS0 = 0                    # gscol10 [10, 1024]
_G3 = 1024                  # g3row6  [6, 1024]
_ETP = 2048                 # etpat10 [10, 40]
_E3P = 2088                 # e3pat6  [6, 20]
_PATW = 2108

_PROGRAM = None


def _build_program(num_devices=NCORES):
    from contextlib import ExitStack

    import concourse.bacc as bacc
    import concourse.mybir as mybir
    import concourse.tile as tile

    f32 = mybir.dt.float32
    f16 = mybir.dt.float16
    bf16 = mybir.dt.bfloat16
    OP = mybir.AluOpType
    EXP = mybir.ActivationFunctionType.Exp

    nc = bacc.Bacc(
        "TRN2",
        target_bir_lowering=False,
        debug=False,
        num_devices=num_devices,
    )

    pw_d = nc.dram_tensor("pw", [20, 800], f32, kind="ExternalInput").ap()
    pats_d = nc.dram_tensor("pats", [10, _PATW], f16, kind="ExternalInput").ap()
    fmat_d = nc.dram_tensor("fmat", [128, NCH], f32, kind="ExternalInput").ap()
    out_d = nc.dram_tensor("out", [8, 128], f32, kind="ExternalOutput").ap()

    with tile.TileContext(nc) as tc, ExitStack() as ctx:
        const = ctx.enter_context(tc.tile_pool(name="const", bufs=1))
        work = ctx.enter_context(tc.tile_pool(name="work", bufs=3))
        psum_pre = ctx.enter_context(
            tc.tile_pool(name="psum_pre", bufs=1, space="PSUM")
        )

        # ---- input loads (3 packed DMAs, dispatched from idle engines) ----
        pats_sb = const.tile([10, _PATW], f16)
        nc.sync.dma_start(out=pats_sb[:], in_=pats_d[:])
        pw_sb = const.tile([20, 800], f32)
        nc.gpsimd.dma_start(out=pw_sb[:], in_=pw_d[:])
        fmat_sb = const.tile([128, NCH], f32)
        nc.scalar.dma_start(out=fmat_sb[:], in_=fmat_d[:])

        # ---- w' = p * Wfac -> bf16 (one fused vector op) ----
        whi = const.tile([20, 400], bf16)
        nc.vector.scalar_tensor_tensor(
            whi[:], pw_sb[:, 0:400], 1.0, pw_sb[:, 400:800],
            op0=OP.mult, op1=OP.mult,
        )

        # ---- E3 [x3, i]: fp16 k=6 exponent matmuls -> Exp -> bf16 ----
        e3s = const.tile([20, NI], bf16)
        for h in range(2):
            sl = slice(h * 512, (h + 1) * 512)
            xp3 = psum_pre.tile([20, 512], f32, tag="xp3", bufs=2)
            nc.tensor.matmul(
                xp3[:],
                lhsT=pats_sb[0:6, _E3P : _E3P + 20],
                rhs=pats_sb[0:6, _G3 + h * 512 : _G3 + h * 512 + 512],
                start=True,
                stop=True,
            )
            nc.scalar.activation(e3s[:, sl], xp3[:], EXP)

        # ---- E1/E2 exponents: fp16 k=10 matmuls, packed [128, 8*40] ----
        xpe = psum_pre.tile([128, NCH * 40], f32)
        for ci in range(NCH):
            nc.tensor.matmul(
                xpe[:, ci * 40 : ci * 40 + 40],
                lhsT=pats_sb[:, ci * 128 : (ci + 1) * 128],
                rhs=pats_sb[:, _ETP : _ETP + 40],
                start=True,
                stop=True,
            )
        et = const.tile([128, NCH * 40], f32)
        nc.scalar.activation(et[:, 0 : 4 * 40], xpe[:, 0 : 4 * 40], EXP)
        nc.scalar.activation(et[:, 4 * 40 : 8 * 40], xpe[:, 4 * 40 : 8 * 40], EXP)

        # ---- e^{F} ----
        ef = const.tile([128, NCH], f32)
        nc.scalar.activation(ef[:], fmat_sb[:], EXP)

        # ---- E12 = E1 (x) E2, bf16, built on GpSimd (keeps DVE free) ----
        e12 = const.tile([128, NCH * 400], bf16)
        for ci in range(NCH):
            e1b = (
                et[:, ci * 40 : ci * 40 + 20]
                .unsqueeze(2)
                .broadcast_to((128, 20, 20))
            )
            e2b = (
                et[:, ci * 40 + 20 : ci * 40 + 40]
                .unsqueeze(1)
                .broadcast_to((128, 20, 20))
            )
            e12v = e12[:, ci * 400 : ci * 400 + 400].rearrange(
                "p (a b) -> p a b", a=20, b=20
            )
            eng = nc.vector if ci < 2 else nc.gpsimd
            eng.tensor_mul(e12v, e1b, e2b)


        # ---- main loop over 8 i-chunks ----
        kdeT = const.tile([128, NCH], f32)
        for ci in range(NCH):
            isl = slice(ci * 128, (ci + 1) * 128)
            o1p = psum_pre.tile([128, 400], f32, tag="o1p", bufs=4)
            nc.tensor.matmul(
                o1p[:], lhsT=e3s[:, isl], rhs=whi[:], start=True, stop=True
            )
            prod = work.tile([128, 400], bf16, tag="prod")
            nc.vector.scalar_tensor_tensor(
                prod[:],
                o1p[:],
                1.0,
                e12[:, ci * 400 : ci * 400 + 400],
                op0=OP.mult,
                op1=OP.mult,
                accum_out=kdeT[:, ci : ci + 1],
            )

        # ---- scale by e^{F}, transpose to row-major i order, DMA out ----
        ksc = const.tile([128, 32], f32)
        nc.vector.memset(ksc[:], 0.0)
        nc.vector.tensor_mul(ksc[:, 0:NCH], kdeT[:], ef[:])
        t32 = const.tile([128, 32], f32)
        nc.vector.transpose(t32[:], ksc[:])
        kout = const.tile([32, 128], f32)
        for r in range(4):
            nc.vector.tensor_copy(
                kout[0:8, r * 32 : (r + 1) * 32], t32[r * 32 : r * 32 + 8, 0:32]
            )
        nc.sync.dma_start(out=out_d[:], in_=kout[0:8, :])

    nc.compile()
    return nc


def _get_program():
    global _PROGRAM
    if _PROGRAM is None:
        _PROGRAM = _build_program()
    return _PROGRAM


def _split16(v):
    hi = v.astype(np.float16).astype(np.float64)
    return hi, v - hi


def _host_inputs(space_probs, cov_inv):
    """Per-core input maps: host-side layout + coordinate-table prep."""
    p = np.asarray(space_probs, dtype=np.float64).reshape(-1)
    A = np.asarray(cov_inv, dtype=np.float64)

    idx = np.indices(GRID, dtype=np.float64).reshape(3, N)
    cc = idx - 9.5                        # centered coords, [3, N]
    c20 = np.arange(20, dtype=np.float64) - 9.5

    G = (cc.T @ A).T                      # [3, N] g_k,i
    q = np.sum(cc * G, axis=0)            # [N]
    s = np.maximum(0.0, 9.5 * np.abs(G) - SHIFT_B)   # [3, N]
    F = -0.5 * q + s.sum(axis=0)          # [N]
    t = 0.5 * np.diag(A)[:, None] * (c20**2)[None, :]  # [3, 20]

    crossexp = -(
        A[0, 1] * cc[0] * cc[1] + A[0, 2] * cc[0] * cc[2] + A[1, 2] * cc[1] * cc[2]
    )
    wfac = np.exp(crossexp).reshape(400, 20).T
    pt = p.reshape(400, 20).T
    pw = np.zeros((20, 800), dtype=np.float32)
    pw[:, 0:400] = pt
    pw[:, 400:800] = wfac

    th = [_split16(t[k]) for k in range(3)]

    # etpat10 rows: [g1h,g1l,g2h,g2l,s1h,s1l,s2h,s2l,1(t hi),1(t lo)]
    etpat = np.zeros((10, 40), dtype=np.float16)
    etpat[0, 0:20] = c20
    etpat[1, 0:20] = c20
    etpat[4, 0:20] = -1.0
    etpat[5, 0:20] = -1.0
    etpat[8, 0:20] = -th[0][0]
    etpat[9, 0:20] = -th[0][1]
    etpat[2, 20:40] = c20
    etpat[3, 20:40] = c20
    etpat[6, 20:40] = -1.0
    etpat[7, 20:40] = -1.0
    etpat[8, 20:40] = -th[1][0]
    etpat[9, 20:40] = -th[1][1]

    # e3pat6 rows: [g3h, g3l, s3h, s3l, 1(t hi), 1(t lo)]
    e3pat = np.zeros((6, 20), dtype=np.float16)
    e3pat[0] = c20
    e3pat[1] = c20
    e3pat[2] = -1.0
    e3pat[3] = -1.0
    e3pat[4] = -th[2][0]
    e3pat[5] = -th[2][1]

    in_maps = []
    for r in range(NCORES):
        i0 = r * ROWS
        sl = slice(i0, i0 + ROWS)

        pats = np.zeros((10, _PATW), dtype=np.float16)
        hi_rows = (0, 2, 4, 6)
        lo_rows = (1, 3, 5, 7)
        for k, src in enumerate((G[0], G[1], s[0], s[1])):
            hi, lo = _split16(src[sl])
            pats[hi_rows[k], _GS0 : _GS0 + ROWS] = hi
            pats[lo_rows[k], _GS0 : _GS0 + ROWS] = lo
        pats[8, _GS0 : _GS0 + ROWS] = 1.0
        pats[9, _GS0 : _GS0 + ROWS] = 1.0

        g3h, g3l = _split16(G[2][sl])
        s3h, s3l = _split16(s[2][sl])
        pats[0, _G3 : _G3 + ROWS] = g3h
        pats[1, _G3 : _G3 + ROWS] = g3l
        pats[2, _G3 : _G3 + ROWS] = s3h
        pats[3, _G3 : _G3 + ROWS] = s3l
        pats[4, _G3 : _G3 + ROWS] = 1.0
        pats[5, _G3 : _G3 + ROWS] = 1.0

        pats[:, _ETP : _ETP + 40] = etpat
        pats[0:6, _E3P : _E3P + 20] = e3pat

        fm = np.zeros((NCH, 128), dtype=np.float32)
        fm.reshape(-1)[:ROWS] = F[sl]
        fmat = np.ascontiguousarray(fm.T)

        in_maps.append({"pw": pw, "pats": pats, "fmat": fmat})
    return in_maps


def kernel(space_probs, cov_inv):
    from concourse.bass_utils import run_bass_kernel_spmd

    nc = _get_program()
    in_maps = _host_inputs(space_probs, cov_inv)
    res = run_bass_kernel_spmd(nc, in_maps, list(range(NCORES)))
    out = np.concatenate(
        [res.results[r]["out"].reshape(-1)[:ROWS] for r in range(NCORES)]
    )
    out = out / out.sum(dtype=np.float64)
    return out.reshape(GRID).astype(np.float32)
